# revision 1
# baseline (speedup 1.0000x reference)
"""Bass/Trainium2 kernel for nn_Attention_21354577395789.

Reference computation (B=16, S=2048, H=1024, D=2H=2048):
    h      = broadcast(hidden[1, 2H]) -> [B, S, 2H]
    cat    = concat([h, enc], -1)                    [B, S, 4H]
    energy = tanh(cat @ attn_w.T + attn_b)           [B, S, H]
    scores = energy @ v_w.T                          [B, S, 1]
    attn   = softmax(scores, axis=1)
    ctx    = attn^T @ enc                            [B, 1, 2H]

Algebraic simplifications:
  * attn_w = [W_h | W_e] along its 4H input dim, so
    cat @ attn_w.T = hidden @ W_h.T + enc @ W_e.T and
    c = hidden @ W_h.T + attn_b is a single [H] vector shared by every
    (b, s). c is computed on the HOST (it is tiny) and uploaded.
  * scores are O(1) in magnitude, so softmax needs no max subtraction:
    u = exp(s) streamed per 512-chunk of S; the device emits
    per-chunk unnormalized context partials and per-chunk exp-sums,
    and the final (sum over chunks) / (sum of exp) happens on host.

Engine placement per 512-column chunk of S:
  PE    : energy matmuls (the only O(S*D*H) work) + v-reduction
  ACT   : tanh(+bias), exp(+chunk sum)
  GpSimd: broadcast exp-weights row across 128 partitions
  Vector: fused multiply+sum of resident encT tiles against the
          broadcast weights (context partials)
enc is loaded from DRAM exactly once (d-major layout only).

Distribution: data-parallel over B across 8 NeuronCores (2 batches per
core), no collectives. Compute in bf16 (fp32 PSUM accumulation).
"""

import os

import numpy as np
import ml_dtypes

B, S, H = 16, 2048, 1024
D = 2 * H          # 2048, encoder feature dim / contraction dim of W_e
N_CORES = 8
BPC = B // N_CORES  # batches per core = 2
NT = 512           # t-chunk (moving-dim) size
KT = D // 128      # 16 k-tiles over the contraction dim d
JT = H // 128      # 8 j-tiles over the energy dim
TBLK = S // NT     # 4 t-chunks per batch

# Per-batch chunk widths. The LAST batch ends with two narrow chunks so
# the post-energy softmax+context tail (which cannot overlap anything)
# is as short as possible.
CHUNKS = [[512] * 4 for _ in range(BPC)]
CHUNKS[BPC - 1] = [512, 512, 512, 256, 256]
NCH = max(len(c) for c in CHUNKS)

BF16 = ml_dtypes.bfloat16

_cache = {}


def _build():
    import concourse.bacc as bacc
    import concourse.tile as tile
    from concourse import mybir

    nc = bacc.Bacc("TRN2", target_bir_lowering=False, debug=False)
    dt = mybir.dt

    # chunk-major: for chunk ci of batch b (width w, t-offset t0), columns
    # [KT*t0 : KT*(t0+w)] hold block[p, kk*w + t] = enc[b, t0+t, kk*128+p],
    # so every chunk DMA is one fully-contiguous run per partition row
    # (long runs -> large DMA packets -> higher effective bandwidth).
    encT2 = nc.declare_dram_parameter(
        "encT2", [BPC, 128, KT * S], dt.bfloat16, isOutput=False
    )
    # w_j2[p, kk*H + jj*128 + j] = w_eT[kk*128 + p, jj*128 + j]
    # (identical layout to the SBUF-resident copy: one contiguous DMA per kk)
    w_j2 = nc.declare_dram_parameter(
        "w_j2", [128, KT * H], dt.bfloat16, isOutput=False
    )
    c_cols_d = nc.declare_dram_parameter("c_cols", [128, JT], dt.float32, isOutput=False)
    v_cols_d = nc.declare_dram_parameter("v_cols", [128, JT], dt.float32, isOutput=False)
    out_part = nc.declare_dram_parameter(
        "out_part", [BPC, 128, NCH * KT], dt.float32, isOutput=True
    )
    out_sums = nc.declare_dram_parameter(
        "out_sums", [BPC, 1, NCH], dt.float32, isOutput=True
    )

    AF = mybir.ActivationFunctionType
    OP = mybir.AluOpType

    with tile.TileContext(nc) as tc:
        with (
            tc.tile_pool(name="weights", bufs=1) as wpool,
            tc.tile_pool(name="enc", bufs=3) as encpool,
            tc.tile_pool(name="energy", bufs=2) as epool,
            tc.tile_pool(name="perb", bufs=2) as bpool,
            tc.tile_pool(name="psum_e", bufs=5, space="PSUM") as pe_pool,
            tc.tile_pool(name="psum_s", bufs=2, space="PSUM") as ps_pool,
        ):
            # ---- resident weights/constants -----------------------------
            # kk-major layout (kk*H + jj*128 + j): each per-kk DMA writes
            # one contiguous 2KB run per partition (256B runs fragment DMA
            # into slow small packets), and the jj=0 energy matmuls can
            # start as soon as the first kk slices land.
            w_sb = wpool.tile([128, KT * H], dt.bfloat16, tag="w")
            c_sb = wpool.tile([128, JT], dt.float32, tag="c")
            v_sb = wpool.tile([128, JT], dt.float32, tag="v")

            def w_stat(kk, jj):
                o = kk * H + jj * 128
                return w_sb[:, o : o + 128]

            def dma_w(kk, half=None):
                lo = kk * H if half != 1 else kk * H + H // 2
                hi = (kk + 1) * H if half != 0 else kk * H + H // 2
                nc.sync.dma_start(
                    w_sb[:, lo:hi], w_j2.ap()[:, lo:hi]
                )

            enc_tiles = {}
            offs = [
                [sum(CHUNKS[b][:ci]) for ci in range(len(CHUNKS[b]))]
                for b in range(BPC)
            ]

            def dma_enc(b, ci, split=False):
                w = CHUNKS[b][ci]
                c0 = KT * offs[b][ci]
                enc_t = encpool.tile(
                    [128, KT * NT], dt.bfloat16, tag="enc", name=f"enc{b}_{ci}"
                )
                src = encT2.ap()[b]
                if split:
                    for k0 in range(0, KT, 2):
                        nc.sync.dma_start(
                            enc_t[:, k0 * w : (k0 + 2) * w],
                            src[:, c0 + k0 * w : c0 + (k0 + 2) * w],
                        )
                else:
                    nc.sync.dma_start(
                        enc_t[:, : KT * w], src[:, c0 : c0 + KT * w]
                    )
                enc_tiles[(b, ci)] = enc_t

            # startup: interleave per-kk stationary slices with the first
            # enc chunk's 4-kk groups so the jj=0 matmuls stream behind
            # the DMA arrivals kk by kk.
            # constants first: the very first tanh needs c_sb, and these
            # 4.5KB cost nothing in bandwidth
            nc.sync.dma_start(c_sb[:], c_cols_d.ap()[:])
            nc.sync.dma_start(v_sb[:], v_cols_d.ap()[:])
            first = encpool.tile([128, KT * NT], dt.bfloat16, tag="enc", name="enc0_0")
            src0 = encT2.ap()[0]
            w00 = CHUNKS[0][0]
            # low jj-halves first: the first chunk's jj=0..3 matmuls need
            # only 2MB of weights; the high halves stream in behind.
            for k0 in range(0, KT, 2):
                dma_w(k0, half=0)
                nc.sync.dma_start(
                    first[:, k0 * w00 : (k0 + 2) * w00],
                    src0[:, k0 * w00 : (k0 + 2) * w00],
                )
                dma_w(k0 + 1, half=0)
            for kk in range(KT):
                dma_w(kk, half=1)
            enc_tiles[(0, 0)] = first
            dma_enc(0, 1)

            sums_t = {}
            part_t = {}
            for b in range(BPC):
                sums_t[b] = bpool.tile(
                    [1, NCH], dt.float32, tag="sums", name=f"sums{b}"
                )
                # per-chunk context partials: part[:, ci*KT + kk]
                part_t[b] = bpool.tile(
                    [128, NCH * KT], dt.float32, tag="part", name=f"part{b}"
                )

            all_chunks = [
                (b, ci) for b in range(BPC) for ci in range(len(CHUNKS[b]))
            ]

            # Scores: the Vector engine pre-multiplies each tanh tile by
            # its v segment (tensor_scalar, 4x mode) and tree-adds the 8
            # products in fp16, so the PE does a SINGLE ones-stationary
            # partition-reduce matmul per chunk instead of 8 v-stationary
            # ones (saves 7/8 of the v-reduction matmul columns). That
            # matmul is deferred into chunk c+1's first energy block so
            # the PE never stalls at a chunk boundary.
            ones_col = wpool.tile([128, 1], dt.float16, tag="ones")
            nc.vector.memset(ones_col[:], 1.0)
            carry = None  # (b, ci, w, s_ps, esum, enc_t)

            def finish_chunk(b, ci, w, s_ps, enc_t):
                # streaming softmax chunk: u = exp(s), chunk sum
                u_row = bpool.tile([1, NT], dt.float16, tag="urow")
                nc.scalar.activation(
                    u_row[:, :w], s_ps[:, :w], AF.Exp,
                    accum_out=sums_t[b][0:1, ci : ci + 1],
                )
                u_bc = bpool.tile([128, NT], dt.float16, tag="ubc")
                nc.gpsimd.partition_broadcast(u_bc[:, :w], u_row[:, :w])
                # context partials: part[:, ci*KT+kk] = sum_t u_t * encT[d, t]
                for kk in range(KT):
                    scratch = bpool.tile([128, NT], dt.bfloat16, tag="scr")
                    nc.vector.scalar_tensor_tensor(
                        out=scratch[:, :w],
                        in0=enc_t[:, kk * w : (kk + 1) * w],
                        scalar=1.0,
                        in1=u_bc[:, :w],
                        op0=OP.mult,
                        op1=OP.mult,
                        accum_out=part_t[b][:, ci * KT + kk : ci * KT + kk + 1],
                    )
                nc.sync.dma_start(
                    out_part.ap()[b][:, ci * KT : (ci + 1) * KT],
                    part_t[b][:, ci * KT : (ci + 1) * KT],
                )
                nch = len(CHUNKS[b])
                if ci == nch - 1:
                    nc.sync.dma_start(
                        out_sums.ap()[b][:, :nch], sums_t[b][0:1, :nch]
                    )

            for b, ci in all_chunks:
                w = CHUNKS[b][ci]
                if (b, ci) not in enc_tiles:
                    dma_enc(b, ci)
                enc_t = enc_tiles.pop((b, ci))
                # prefetch next chunk right away
                nch = len(CHUNKS[b])
                nb, nci = (b, ci + 1) if ci + 1 < nch else (b + 1, 0)
                if nb < BPC and (nb, nci) not in enc_tiles:
                    dma_enc(nb, nci)

                s_ps = ps_pool.tile(
                    [1, NT], dt.float32, tag="sps", name=f"sps{b}_{ci}"
                )
                e_all = epool.tile(
                    [128, JT * NT], dt.bfloat16, tag="eall", name=f"eall{b}_{ci}"
                )
                acc = None
                for jj in range(JT):
                    e_ps = pe_pool.tile([128, NT], dt.float32, tag="eps")
                    for kk in range(KT):
                        nc.tensor.matmul(
                            e_ps[:, :w],
                            w_stat(kk, jj),
                            enc_t[:, kk * w : (kk + 1) * w],
                            start=(kk == 0),
                            stop=(kk == KT - 1),
                        )
                    if jj == 0 and carry is not None:
                        pb, pci, pw, ps_ps, pesum, penc_t = carry
                        nc.tensor.matmul(
                            ps_ps[:, :pw], ones_col[:], pesum[:, :pw],
                            start=True, stop=True,
                        )
                        finish_chunk(pb, pci, pw, ps_ps, penc_t)
                        carry = None
                    nc.scalar.activation(
                        e_all[:, jj * NT : jj * NT + w], e_ps[:, :w], AF.Tanh,
                        bias=c_sb[:, jj : jj + 1],
                    )
                    ev = bpool.tile(
                        [128, NT], dt.float16, tag="ev", bufs=3, name=f"ev{jj}"
                    )
                    nc.vector.tensor_scalar_mul(
                        ev[:, :w], e_all[:, jj * NT : jj * NT + w],
                        v_sb[:, jj : jj + 1],
                    )
                    if acc is None:
                        acc = ev
                    else:
                        nacc = bpool.tile(
                            [128, NT], dt.float16, tag="esum", bufs=3,
                            name=f"esum{jj}",
                        )
                        nc.vector.tensor_add(nacc[:, :w], acc[:, :w], ev[:, :w])
                        acc = nacc
                carry = (b, ci, w, s_ps, acc, enc_t)

            # drain the final chunk
            pb, pci, pw, ps_ps, pesum, penc_t = carry
            nc.tensor.matmul(
                ps_ps[:, :pw], ones_col[:], pesum[:, :pw], start=True, stop=True
            )
            finish_chunk(pb, pci, pw, ps_ps, penc_t)

    nc.compile()
    return nc


def _get_nc():
    if "nc" not in _cache:
        import time

        t0 = time.time()
        _cache["nc"] = _build()
        if os.environ.get("KERNEL_TRACE"):
            print(f"[kernel] bass build+compile: {time.time() - t0:.1f} s")
    return _cache["nc"]


def kernel(hidden, encoder_outputs, attn_w, attn_b, v_w):
    from concourse.bass_utils import run_bass_kernel_spmd

    nc = _get_nc()

    hidden = np.asarray(hidden, dtype=np.float32)
    enc = np.asarray(encoder_outputs, dtype=np.float32)
    attn_w = np.asarray(attn_w, dtype=np.float32)
    attn_b = np.asarray(attn_b, dtype=np.float32)
    v_w = np.asarray(v_w, dtype=np.float32)

    w_eT = np.ascontiguousarray(attn_w[:, D:].T)                 # [D, H]
    # (kk, p, jh) -> (p, kk, jh): same layout as the SBUF-resident copy
    w_j2 = np.ascontiguousarray(
        w_eT.reshape(KT, 128, H).transpose(1, 0, 2).reshape(128, KT * H)
    ).astype(BF16)
    c = (hidden @ attn_w[:, :D].T + attn_b).astype(np.float32)   # [1, H]
    c_cols = np.ascontiguousarray(c.reshape(JT, 128).T)          # [128, JT]
    v_cols = np.ascontiguousarray(v_w.reshape(JT, 128).T)

    in_maps = []
    for cidx in range(N_CORES):
        sl = enc[cidx * BPC : (cidx + 1) * BPC]                  # [BPC, S, D]
        rows = []
        for b in range(BPC):
            t0 = 0
            blocks = []
            for wdt in CHUNKS[b]:
                blk = (
                    sl[b, t0 : t0 + wdt]
                    .reshape(wdt, KT, 128)
                    .transpose(2, 1, 0)
                    .reshape(128, KT * wdt)
                )
                blocks.append(blk)
                t0 += wdt
            rows.append(np.concatenate(blocks, axis=1))
        encT2 = np.ascontiguousarray(np.stack(rows)).astype(BF16)
        in_maps.append(
            {"encT2": encT2, "w_j2": w_j2, "c_cols": c_cols, "v_cols": v_cols}
        )

    trace = bool(os.environ.get("KERNEL_TRACE"))
    if trace:
        _install_prof_shim()
    res = run_bass_kernel_spmd(
        nc, in_maps, core_ids=list(range(N_CORES)), trace=trace
    )
    if trace:
        _cache["last_exec_time_ns"] = res.exec_time_ns
        print(f"HW exec time: {res.exec_time_ns} ns")

    ctx = np.empty((B, 1, D), dtype=np.float32)
    for cidx in range(N_CORES):
        part = np.asarray(res.results[cidx]["out_part"], dtype=np.float32)
        sums = np.asarray(res.results[cidx]["out_sums"], dtype=np.float32)
        for b in range(BPC):
            nch = len(CHUNKS[b])
            acc = part[b][:, : nch * KT].reshape(128, nch, KT).sum(axis=1)
            ctx[cidx * BPC + b, 0, :] = (
                acc / sums[b][0, :nch].sum()
            ).T.reshape(D)
    return ctx


def _install_prof_shim():
    """antenv.axon_hooks is absent from this image; inject it so
    run_bass_kernel_spmd(trace=True) can capture NTFF profiles."""
    import sys
    import types

    if "antenv.axon_hooks" in sys.modules:
        return
    import antenv

    mod = types.ModuleType("antenv.axon_hooks")
    mod._hook = None
    mod.set_axon_ntff_profile_hook = lambda h: setattr(mod, "_hook", h)
    mod.get_axon_ntff_profile_hook = lambda: mod._hook
    sys.modules["antenv.axon_hooks"] = mod
    antenv.axon_hooks = mod
    try:
        from trn_agent_boot.trn_boot import _ntff_profile_via_ctypes

        mod.set_axon_ntff_profile_hook(
            _ntff_profile_via_ctypes("/opt/axon/libaxon_pjrt.so")
        )
    except Exception:
        pass



# revision 2
# speedup vs baseline: 1.3604x; 1.3604x over previous
"""Bass/Trainium2 kernel for nn_Attention_21354577395789 (fp8 DoubleRow).

Reference computation (B=16, S=2048, H=1024, D=2H=2048):
    h      = broadcast(hidden[1, 2H]) -> [B, S, 2H]
    cat    = concat([h, enc], -1)                    [B, S, 4H]
    energy = tanh(cat @ attn_w.T + attn_b)           [B, S, H]
    scores = energy @ v_w.T                          [B, S, 1]
    attn   = softmax(scores, axis=1)
    ctx    = attn^T @ enc                            [B, 1, 2H]

Algebraic simplifications (as the bf16 baseline):
  * attn_w = [W_h | W_e]; c = hidden @ W_h.T + attn_b is a single [H]
    vector computed on the host.
  * streaming softmax without max subtraction; per-chunk unnormalized
    context partials + exp-sums; final reduction on host.

Precision strategy (the speedup over the bf16 baseline):
  * The O(S*D*H) energy matmul runs in fp8-e4m3 with
    perf_mode=DoubleRow: two k-slices (256 contraction rows) per PE
    pass -> ~1.8x the bf16 matmul rate. W_e is pre-scaled by WS=64 on
    the host (its std 1/64 would land in e4m3's subnormal range);
    the tanh activation applies scale=1/WS to undo it.
  * Optionally the last NKK16 k-tiles run as plain fp16 matmuls into
    the same PSUM accumulation, dialing the quantization error down
    (sim: NKK16=0 -> 1.78e-2, 2 -> 1.68e-2, 4 -> 1.55e-2 vs the 2e-2
    gate) at ~267ns per extra matmul.
  * The context path (DVE) reads a separate fp16 copy of enc, so fp8
    noise never touches the context accumulation.

Engine placement per chunk of S (as baseline): PE energy matmuls +
ones-reduce; ACT tanh/exp; GpSimd broadcast; DVE v-mult tree + context
partials. Data-parallel over B across 8 cores, no collectives.
"""

import os

import numpy as np
import ml_dtypes

B, S, H = 16, 2048, 1024
D = 2 * H
N_CORES = 8
BPC = B // N_CORES  # 2
NT = 512
KT = D // 128       # 16 k-tiles
JT = H // 128       # 8 j-tiles

WS = 64.0           # host-side W_e scale (undone in tanh's scale arg)
NKK16 = 0           # trailing k-tiles computed in fp16 (error dial)

CHUNKS = [[512] * 4 for _ in range(BPC)]
CHUNKS[BPC - 1] = [512, 512, 512, 256, 256]
NCH = max(len(c) for c in CHUNKS)

F8 = ml_dtypes.float8_e4m3
F16 = np.float16

_cache = {}


def _build():
    import concourse.bacc as bacc
    import concourse.tile as tile
    from concourse import mybir

    nc = bacc.Bacc("TRN2", target_bir_lowering=False, debug=False)
    dt = mybir.dt
    DR = mybir.MatmulPerfMode.DoubleRow

    # chunk-major: for chunk ci of batch b (width w, t-offset t0), columns
    # [KT*t0 : KT*(t0+w)] hold block[p, kk*w + t] = enc[b, t0+t, kk*128+p]
    enc8_d = nc.declare_dram_parameter(
        "enc8", [BPC, 128, KT * S], dt.float8e4, isOutput=False
    )
    ench_d = nc.declare_dram_parameter(
        "ench", [BPC, 128, KT * S], dt.float16, isOutput=False
    )
    # w8[p, kk*H + jj*128 + j] = WS * w_eT[kk*128 + p, jj*128 + j]
    w8_d = nc.declare_dram_parameter("w8", [128, KT * H], dt.float8e4, isOutput=False)
    if NKK16:
        wh_d = nc.declare_dram_parameter(
            "wh", [128, NKK16 * H], dt.float16, isOutput=False
        )
    c_cols_d = nc.declare_dram_parameter("c_cols", [128, JT], dt.float32, isOutput=False)
    v_cols_d = nc.declare_dram_parameter("v_cols", [128, JT], dt.float32, isOutput=False)
    out_part = nc.declare_dram_parameter(
        "out_part", [BPC, 128, NCH * KT], dt.float32, isOutput=True
    )
    out_sums = nc.declare_dram_parameter(
        "out_sums", [BPC, 1, NCH], dt.float32, isOutput=True
    )

    AF = mybir.ActivationFunctionType
    OP = mybir.AluOpType
    NP8 = KT - NKK16          # k-tiles in fp8 (paired for DoubleRow)
    NPAIR = NP8 // 2

    with tile.TileContext(nc) as tc:
        with (
            tc.tile_pool(name="weights", bufs=1) as wpool,
            tc.tile_pool(name="enc8", bufs=3) as e8pool,
            tc.tile_pool(name="ench", bufs=3) as ehpool,
            tc.tile_pool(name="energy", bufs=2) as epool,
            tc.tile_pool(name="perb", bufs=2) as bpool,
            tc.tile_pool(name="psum_e", bufs=5, space="PSUM") as pe_pool,
            tc.tile_pool(name="psum_s", bufs=2, space="PSUM") as ps_pool,
        ):
            # ---- resident weights/constants -----------------------------
            w8_sb = wpool.tile([128, KT, H], dt.float8e4, tag="w8")
            if NKK16:
                wh_sb = wpool.tile([128, NKK16, H], dt.float16, tag="wh")
            c_sb = wpool.tile([128, JT], dt.float32, tag="c")
            v_sb = wpool.tile([128, JT], dt.float32, tag="v")

            def dma_w8(kk, half=None):
                lo = 0 if half != 1 else H // 2
                hi = H if half != 0 else H // 2
                nc.sync.dma_start(
                    w8_sb[:, kk, lo:hi], w8_d.ap()[:, kk * H + lo : kk * H + hi]
                )

            enc_tiles = {}
            offs = [
                [sum(CHUNKS[b][:ci]) for ci in range(len(CHUNKS[b]))]
                for b in range(BPC)
            ]

            def dma_enc(b, ci, which):
                """DMA one chunk of enc8 ('8') or ench ('h')."""
                w = CHUNKS[b][ci]
                c0 = KT * offs[b][ci]
                pool, dram, dtp = (
                    (e8pool, enc8_d, dt.float8e4)
                    if which == "8"
                    else (ehpool, ench_d, dt.float16)
                )
                t = pool.tile(
                    [128, KT, NT], dtp, tag="enc" + which, name=f"enc{which}{b}_{ci}"
                )
                src = dram.ap()[b]
                if w == NT:
                    nc.sync.dma_start(t[:, :, :], src[:, c0 : c0 + KT * NT])
                else:
                    for kk in range(KT):
                        nc.sync.dma_start(
                            t[:, kk, :w], src[:, c0 + kk * w : c0 + (kk + 1) * w]
                        )
                enc_tiles[(b, ci, which)] = t

            # startup: constants, then interleave per-kk fp8 weight slices
            # with the first chunk's enc8 pair-blocks so the jj-low matmuls
            # stream kk by kk behind the DMA arrivals.
            nc.sync.dma_start(c_sb[:], c_cols_d.ap()[:])
            nc.sync.dma_start(v_sb[:], v_cols_d.ap()[:])
            first8 = e8pool.tile([128, KT, NT], dt.float8e4, tag="enc8", name="enc80_0")
            src80 = enc8_d.ap()[0]
            for k0 in range(0, KT, 2):
                dma_w8(k0, half=0)
                nc.sync.dma_start(
                    first8[:, k0 : k0 + 2, :],
                    src80[:, k0 * NT : (k0 + 2) * NT],
                )
                dma_w8(k0 + 1, half=0)
            if NKK16:
                nc.sync.dma_start(wh_sb[:, :, :], wh_d.ap()[:, :])
            for kk in range(KT):
                dma_w8(kk, half=1)
            enc_tiles[(0, 0, "8")] = first8
            dma_enc(0, 0, "h")
            dma_enc(0, 1, "8")
            dma_enc(0, 1, "h")

            sums_t = {}
            part_t = {}
            for b in range(BPC):
                sums_t[b] = bpool.tile(
                    [1, NCH], dt.float32, tag="sums", name=f"sums{b}"
                )
                part_t[b] = bpool.tile(
                    [128, NCH * KT], dt.float32, tag="part", name=f"part{b}"
                )

            all_chunks = [
                (b, ci) for b in range(BPC) for ci in range(len(CHUNKS[b]))
            ]

            ones_col = wpool.tile([128, 1], dt.float16, tag="ones")
            nc.vector.memset(ones_col[:], 1.0)
            carry = None  # (b, ci, w, s_ps, esum, ench_t)

            def finish_chunk(b, ci, w, s_ps, ench_t):
                u_row = bpool.tile([1, NT], dt.float16, tag="urow")
                nc.scalar.activation(
                    u_row[:, :w], s_ps[:, :w], AF.Exp,
                    accum_out=sums_t[b][0:1, ci : ci + 1],
                )
                u_bc = bpool.tile([128, NT], dt.float16, tag="ubc")
                nc.gpsimd.partition_broadcast(u_bc[:, :w], u_row[:, :w])
                for kk in range(KT):
                    scratch = bpool.tile([128, NT], dt.bfloat16, tag="scr")
                    nc.vector.scalar_tensor_tensor(
                        out=scratch[:, :w],
                        in0=ench_t[:, kk, :w],
                        scalar=1.0,
                        in1=u_bc[:, :w],
                        op0=OP.mult,
                        op1=OP.mult,
                        accum_out=part_t[b][:, ci * KT + kk : ci * KT + kk + 1],
                    )
                nc.sync.dma_start(
                    out_part.ap()[b][:, ci * KT : (ci + 1) * KT],
                    part_t[b][:, ci * KT : (ci + 1) * KT],
                )
                nch = len(CHUNKS[b])
                if ci == nch - 1:
                    nc.sync.dma_start(
                        out_sums.ap()[b][:, :nch], sums_t[b][0:1, :nch]
                    )

            for b, ci in all_chunks:
                w = CHUNKS[b][ci]
                for which in ("8", "h"):
                    if (b, ci, which) not in enc_tiles:
                        dma_enc(b, ci, which)
                enc8_t = enc_tiles.pop((b, ci, "8"))
                ench_t = enc_tiles.pop((b, ci, "h"))
                # prefetch next chunk
                nch = len(CHUNKS[b])
                nb, nci = (b, ci + 1) if ci + 1 < nch else (b + 1, 0)
                if nb < BPC:
                    for which in ("8", "h"):
                        if (nb, nci, which) not in enc_tiles:
                            dma_enc(nb, nci, which)

                s_ps = ps_pool.tile(
                    [1, NT], dt.float32, tag="sps", name=f"sps{b}_{ci}"
                )
                e_all = epool.tile(
                    [128, JT * NT], dt.bfloat16, tag="eall", name=f"eall{b}_{ci}"
                )
                acc = None
                for jj in range(JT):
                    e_ps = pe_pool.tile([128, NT], dt.float32, tag="eps")
                    for pk in range(NPAIR):
                        kk0 = 2 * pk
                        nc.tensor.matmul(
                            e_ps[:, :w],
                            w8_sb[:, kk0 : kk0 + 2, jj * 128 : (jj + 1) * 128],
                            enc8_t[:, kk0 : kk0 + 2, :w],
                            start=(pk == 0),
                            stop=(pk == NPAIR - 1 and NKK16 == 0),
                            perf_mode=DR,
                        )
                    for i in range(NKK16):
                        kk = NP8 + i
                        nc.tensor.matmul(
                            e_ps[:, :w],
                            wh_sb[:, i, jj * 128 : (jj + 1) * 128],
                            ench_t[:, kk, :w],
                            start=False,
                            stop=(i == NKK16 - 1),
                        )
                    if jj == 0 and carry is not None:
                        pb, pci, pw, ps_ps, pesum, pench_t = carry
                        nc.tensor.matmul(
                            ps_ps[:, :pw], ones_col[:], pesum[:, :pw],
                            start=True, stop=True,
                        )
                        finish_chunk(pb, pci, pw, ps_ps, pench_t)
                        carry = None
                    nc.scalar.activation(
                        e_all[:, jj * NT : jj * NT + w], e_ps[:, :w], AF.Tanh,
                        bias=c_sb[:, jj : jj + 1], scale=1.0 / WS,
                    )
                    ev = bpool.tile(
                        [128, NT], dt.float16, tag="ev", bufs=3, name=f"ev{jj}"
                    )
                    nc.vector.tensor_scalar_mul(
                        ev[:, :w], e_all[:, jj * NT : jj * NT + w],
                        v_sb[:, jj : jj + 1],
                    )
                    if acc is None:
                        acc = ev
                    else:
                        nacc = bpool.tile(
                            [128, NT], dt.float16, tag="esum", bufs=3,
                            name=f"esum{jj}",
                        )
                        nc.vector.tensor_add(nacc[:, :w], acc[:, :w], ev[:, :w])
                        acc = nacc
                carry = (b, ci, w, s_ps, acc, ench_t)

            pb, pci, pw, ps_ps, pesum, pench_t = carry
            nc.tensor.matmul(
                ps_ps[:, :pw], ones_col[:], pesum[:, :pw], start=True, stop=True
            )
            finish_chunk(pb, pci, pw, ps_ps, pench_t)

    nc.compile()
    return nc


def _get_nc():
    if "nc" not in _cache:
        import time

        t0 = time.time()
        _cache["nc"] = _build()
        if os.environ.get("KERNEL_TRACE"):
            print(f"[kernel] bass build+compile: {time.time() - t0:.1f} s")
    return _cache["nc"]


def kernel(hidden, encoder_outputs, attn_w, attn_b, v_w):
    from concourse.bass_utils import run_bass_kernel_spmd

    nc = _get_nc()

    hidden = np.asarray(hidden, dtype=np.float32)
    enc = np.asarray(encoder_outputs, dtype=np.float32)
    attn_w = np.asarray(attn_w, dtype=np.float32)
    attn_b = np.asarray(attn_b, dtype=np.float32)
    v_w = np.asarray(v_w, dtype=np.float32)

    w_eT = np.ascontiguousarray(attn_w[:, D:].T) * WS            # [D, H]
    w_kk = w_eT.reshape(KT, 128, H).transpose(1, 0, 2)           # [128, KT, H]
    w8 = np.ascontiguousarray(w_kk).reshape(128, KT * H).astype(F8)
    if NKK16:
        wh = np.ascontiguousarray(w_kk[:, KT - NKK16 :]).reshape(
            128, NKK16 * H
        ).astype(F16)
    c = (hidden @ attn_w[:, :D].T + attn_b).astype(np.float32)   # [1, H]
    c_cols = np.ascontiguousarray(c.reshape(JT, 128).T)          # [128, JT]
    v_cols = np.ascontiguousarray(v_w.reshape(JT, 128).T)

    in_maps = []
    for cidx in range(N_CORES):
        sl = enc[cidx * BPC : (cidx + 1) * BPC]                  # [BPC, S, D]
        rows = []
        for b in range(BPC):
            t0 = 0
            blocks = []
            for wdt in CHUNKS[b]:
                blk = (
                    sl[b, t0 : t0 + wdt]
                    .reshape(wdt, KT, 128)
                    .transpose(2, 1, 0)
                    .reshape(128, KT * wdt)
                )
                blocks.append(blk)
                t0 += wdt
            rows.append(np.concatenate(blocks, axis=1))
        encT2 = np.ascontiguousarray(np.stack(rows))
        m = {
            "enc8": encT2.astype(F8),
            "ench": encT2.astype(F16),
            "w8": w8,
            "c_cols": c_cols,
            "v_cols": v_cols,
        }
        if NKK16:
            m["wh"] = wh
        in_maps.append(m)

    trace = bool(os.environ.get("KERNEL_TRACE"))
    if trace:
        _install_prof_shim()
    res = run_bass_kernel_spmd(
        nc, in_maps, core_ids=list(range(N_CORES)), trace=trace
    )
    if trace:
        _cache["last_exec_time_ns"] = res.exec_time_ns
        print(f"HW exec time: {res.exec_time_ns} ns")

    ctx = np.empty((B, 1, D), dtype=np.float32)
    for cidx in range(N_CORES):
        part = np.asarray(res.results[cidx]["out_part"], dtype=np.float32)
        sums = np.asarray(res.results[cidx]["out_sums"], dtype=np.float32)
        for b in range(BPC):
            nch = len(CHUNKS[b])
            acc = part[b][:, : nch * KT].reshape(128, nch, KT).sum(axis=1)
            ctx[cidx * BPC + b, 0, :] = (
                acc / sums[b][0, :nch].sum()
            ).T.reshape(D)
    return ctx


def _install_prof_shim():
    """antenv.axon_hooks is absent from this image; inject it so
    run_bass_kernel_spmd(trace=True) can capture NTFF profiles."""
    import sys
    import types

    if "antenv.axon_hooks" in sys.modules:
        return
    import antenv

    mod = types.ModuleType("antenv.axon_hooks")
    mod._hook = None
    mod.set_axon_ntff_profile_hook = lambda h: setattr(mod, "_hook", h)
    mod.get_axon_ntff_profile_hook = lambda: mod._hook
    sys.modules["antenv.axon_hooks"] = mod
    antenv.axon_hooks = mod
    try:
        from trn_agent_boot.trn_boot import _ntff_profile_via_ctypes

        mod.set_axon_ntff_profile_hook(
            _ntff_profile_via_ctypes("/opt/axon/libaxon_pjrt.so")
        )
    except Exception:
        pass


# revision 9
# speedup vs baseline: 1.5742x; 1.1572x over previous
"""Bass/Trainium2 kernel for nn_Attention_21354577395789 (fp8 DoubleRow).

Reference computation (B=16, S=2048, H=1024, D=2H=2048):
    h      = broadcast(hidden[1, 2H]) -> [B, S, 2H]
    cat    = concat([h, enc], -1)                    [B, S, 4H]
    energy = tanh(cat @ attn_w.T + attn_b)           [B, S, H]
    scores = energy @ v_w.T                          [B, S, 1]
    attn   = softmax(scores, axis=1)
    ctx    = attn^T @ enc                            [B, 1, 2H]

Algebraic simplifications (as the bf16 baseline):
  * attn_w = [W_h | W_e]; c = hidden @ W_h.T + attn_b is a single [H]
    vector computed on the host.
  * streaming softmax without max subtraction; per-chunk unnormalized
    context partials + exp-sums; final reduction on host.

Precision strategy (the speedup over the bf16 baseline):
  * The O(S*D*H) energy matmul runs in fp8-e4m3 with
    perf_mode=DoubleRow: two k-slices (256 contraction rows) per PE
    pass -> ~1.8x the bf16 matmul rate. W_e is pre-scaled by WS=64 on
    the host (its std 1/64 would land in e4m3's subnormal range);
    the tanh activation applies scale=1/WS to undo it.
  * Optionally the last NKK16 k-tiles run as plain fp16 matmuls into
    the same PSUM accumulation, dialing the quantization error down
    (sim: NKK16=0 -> 1.78e-2, 2 -> 1.68e-2, 4 -> 1.55e-2 vs the 2e-2
    gate) at ~267ns per extra matmul.
  * The context path (DVE) reads a separate fp16 copy of enc, so fp8
    noise never touches the context accumulation.

Engine placement per chunk of S (as baseline): PE energy matmuls +
ones-reduce; ACT tanh/exp; GpSimd broadcast; DVE v-mult tree + context
partials. Data-parallel over B across 8 cores, no collectives.
"""

import os

import numpy as np
import ml_dtypes

B, S, H = 16, 2048, 1024
D = 2 * H
N_CORES = 8
BPC = B // N_CORES  # 2
NT = 512
KT = D // 128       # 16 k-tiles
JT = H // 128       # 8 j-tiles

WS = 64.0           # host-side W_e scale (undone in tanh's scale arg)
NKK16 = 0           # trailing k-tiles computed in fp16 (error dial)

CHUNKS = [[512] * 4 for _ in range(BPC)]
CHUNKS[BPC - 1] = [512, 512, 512, 256, 256]
NCH = max(len(c) for c in CHUNKS)

F8 = ml_dtypes.float8_e4m3
F16 = np.float16

_cache = {}


def _build():
    import concourse.bacc as bacc
    import concourse.tile as tile
    from concourse import mybir

    nc = bacc.Bacc("TRN2", target_bir_lowering=False, debug=False)
    dt = mybir.dt
    DR = mybir.MatmulPerfMode.DoubleRow

    # chunk-major: for chunk ci of batch b (width w, t-offset t0), columns
    # [KT*t0 : KT*(t0+w)] hold block[p, kk*w + t] = enc[b, t0+t, kk*128+p]
    # enc8 feeds the PE (3D tile for the DoubleRow pair APs); ench feeds
    # the DVE/GpSimd context path (2D tile -- 3D APs cost ~+220ns per
    # DVE op in the reshape front-end) and the optional fp16 matmuls.
    enc8_d = nc.declare_dram_parameter(
        "enc8", [BPC, 128, KT * S], dt.float8e4, isOutput=False
    )
    ench_d = nc.declare_dram_parameter(
        "ench", [BPC, 128, KT * S], dt.float16, isOutput=False
    )
    # w8[p, kk*H + jj*128 + j] = WS * w_eT[kk*128 + p, jj*128 + j]
    w8_d = nc.declare_dram_parameter("w8", [128, KT * H], dt.float8e4, isOutput=False)
    if NKK16:
        wh_d = nc.declare_dram_parameter(
            "wh", [128, NKK16 * H], dt.float16, isOutput=False
        )
    c_cols_d = nc.declare_dram_parameter("c_cols", [128, JT], dt.float32, isOutput=False)
    v_cols_d = nc.declare_dram_parameter("v_cols", [128, JT], dt.float32, isOutput=False)
    out_part = nc.declare_dram_parameter(
        "out_part", [BPC, 128, NCH * KT], dt.float32, isOutput=True
    )
    out_sums = nc.declare_dram_parameter(
        "out_sums", [BPC, 1, NCH], dt.float32, isOutput=True
    )

    AF = mybir.ActivationFunctionType
    OP = mybir.AluOpType
    NP8 = KT - NKK16          # k-tiles in fp8 (paired for DoubleRow)
    NPAIR = NP8 // 2

    with tile.TileContext(nc) as tc:
        with (
            tc.tile_pool(name="weights", bufs=1) as wpool,
            tc.tile_pool(name="enc8", bufs=3) as e8pool,
            tc.tile_pool(name="ench", bufs=3) as ehpool,
            tc.tile_pool(name="energy", bufs=2) as epool,
            tc.tile_pool(name="perb", bufs=2) as bpool,
            tc.tile_pool(name="psum_e", bufs=5, space="PSUM") as pe_pool,
            tc.tile_pool(name="psum_s", bufs=2, space="PSUM") as ps_pool,
        ):
            # ---- resident weights/constants -----------------------------
            w8_sb = wpool.tile([128, KT, H], dt.float8e4, tag="w8")
            if NKK16:
                wh_sb = wpool.tile([128, NKK16, H], dt.float16, tag="wh")
            c_sb = wpool.tile([128, JT], dt.float32, tag="c")
            v_sb = wpool.tile([128, JT], dt.float32, tag="v")

            def dma_w8(kk, half=None):
                lo = 0 if half != 1 else H // 2
                hi = H if half != 0 else H // 2
                nc.sync.dma_start(
                    w8_sb[:, kk, lo:hi], w8_d.ap()[:, kk * H + lo : kk * H + hi]
                )

            enc_tiles = {}
            offs = [
                [sum(CHUNKS[b][:ci]) for ci in range(len(CHUNKS[b]))]
                for b in range(BPC)
            ]

            def dma_enc(b, ci, which):
                """DMA one chunk of enc8 ('8', 3D tile) or ench ('h', 2D)."""
                w = CHUNKS[b][ci]
                c0 = KT * offs[b][ci]
                src = (enc8_d if which == "8" else ench_d).ap()[b]
                if which == "8":
                    t = e8pool.tile(
                        [128, KT, NT], dt.float8e4, tag="enc8", name=f"enc8{b}_{ci}"
                    )
                    if w == NT:
                        nc.sync.dma_start(t[:, :, :], src[:, c0 : c0 + KT * NT])
                    else:
                        for k0 in range(0, KT, 2):
                            nc.sync.dma_start(
                                t[:, k0 : k0 + 2, :w],
                                src[:, c0 + k0 * w : c0 + (k0 + 2) * w],
                            )
                else:
                    # 2D chunk-major [kk*w + t], exactly the DRAM layout
                    t = ehpool.tile(
                        [128, KT * NT], dt.float16, tag="ench", name=f"ench{b}_{ci}"
                    )
                    nc.sync.dma_start(t[:, : KT * w], src[:, c0 : c0 + KT * w])
                enc_tiles[(b, ci, which)] = t

            # startup: constants, then interleave per-kk fp8 weight slices
            # with the first chunk's enc8 pair-blocks so the jj-low matmuls
            # stream kk by kk behind the DMA arrivals.
            nc.sync.dma_start(c_sb[:], c_cols_d.ap()[:])
            nc.sync.dma_start(v_sb[:], v_cols_d.ap()[:])
            first8 = e8pool.tile([128, KT, NT], dt.float8e4, tag="enc8", name="enc80_0")
            src80 = enc8_d.ap()[0]
            for k0 in range(0, KT, 2):
                dma_w8(k0, half=0)
                nc.sync.dma_start(
                    first8[:, k0 : k0 + 2, :],
                    src80[:, k0 * NT : (k0 + 2) * NT],
                )
                dma_w8(k0 + 1, half=0)
            if NKK16:
                nc.sync.dma_start(wh_sb[:, :, :], wh_d.ap()[:, :])
            for kk in range(KT):
                dma_w8(kk, half=1)
            enc_tiles[(0, 0, "8")] = first8
            dma_enc(0, 1, "8")
            dma_enc(0, 0, "h")

            sums_t = {}
            part_t = {}
            for b in range(BPC):
                sums_t[b] = bpool.tile(
                    [1, NCH], dt.float32, tag="sums", name=f"sums{b}"
                )
                part_t[b] = bpool.tile(
                    [128, NCH * KT], dt.float32, tag="part", name=f"part{b}"
                )

            all_chunks = [
                (b, ci) for b in range(BPC) for ci in range(len(CHUNKS[b]))
            ]

            ones_col = wpool.tile([128, 1], dt.float16, tag="ones")
            nc.vector.memset(ones_col[:], 1.0)
            carry = None  # (b, ci, w, s_ps, esum, ench_t)

            def finish_chunk(b, ci, w, s_ps, ench_t):
                u_row = bpool.tile([1, NT], dt.float16, tag="urow")
                nc.scalar.activation(
                    u_row[:, :w], s_ps[:, :w], AF.Exp,
                    accum_out=sums_t[b][0:1, ci : ci + 1],
                )
                u_bc = bpool.tile([128, NT], dt.float16, tag="ubc")
                nc.gpsimd.partition_broadcast(u_bc[:, :w], u_row[:, :w])
                for kk in range(KT):
                    eng = nc.vector
                    scratch = bpool.tile([128, NT], dt.bfloat16, tag="scr")
                    eng.scalar_tensor_tensor(
                        out=scratch[:, :w],
                        in0=ench_t[:, kk * w : (kk + 1) * w],
                        scalar=1.0,
                        in1=u_bc[:, :w],
                        op0=OP.mult,
                        op1=OP.mult,
                        accum_out=part_t[b][:, ci * KT + kk : ci * KT + kk + 1],
                    )
                nc.sync.dma_start(
                    out_part.ap()[b][:, ci * KT : (ci + 1) * KT],
                    part_t[b][:, ci * KT : (ci + 1) * KT],
                )
                nch = len(CHUNKS[b])
                if ci == nch - 1:
                    nc.sync.dma_start(
                        out_sums.ap()[b][:, :nch], sums_t[b][0:1, :nch]
                    )

            for b, ci in all_chunks:
                w = CHUNKS[b][ci]
                for which in ("8", "h"):
                    if (b, ci, which) not in enc_tiles:
                        dma_enc(b, ci, which)
                enc8_t = enc_tiles.pop((b, ci, "8"))
                ench_t = enc_tiles.pop((b, ci, "h"))
                # prefetch next chunk: enc8 now (feeds the PE at the next
                # chunk boundary); ench at jj=4 (not needed until that
                # chunk's finish_chunk, one chunk later)
                nch = len(CHUNKS[b])
                nb, nci = (b, ci + 1) if ci + 1 < nch else (b + 1, 0)
                prefetch = (nb, nci) if nb < BPC else None
                if prefetch and (nb, nci, "8") not in enc_tiles:
                    dma_enc(nb, nci, "8")

                s_ps = ps_pool.tile(
                    [1, NT], dt.float32, tag="sps", name=f"sps{b}_{ci}"
                )
                e_all = epool.tile(
                    [128, JT * NT], dt.bfloat16, tag="eall", name=f"eall{b}_{ci}"
                )
                acc = None
                for jj in range(JT):
                    e_ps = pe_pool.tile([128, NT], dt.float32, tag="eps")
                    for pk in range(NPAIR):
                        kk0 = 2 * pk
                        nc.tensor.matmul(
                            e_ps[:, :w],
                            w8_sb[:, kk0 : kk0 + 2, jj * 128 : (jj + 1) * 128],
                            enc8_t[:, kk0 : kk0 + 2, :w],
                            start=(pk == 0),
                            stop=(pk == NPAIR - 1 and NKK16 == 0),
                            perf_mode=DR,
                        )
                    for i in range(NKK16):
                        kk = NP8 + i
                        nc.tensor.matmul(
                            e_ps[:, :w],
                            wh_sb[:, i, jj * 128 : (jj + 1) * 128],
                            ench_t[:, kk * w : (kk + 1) * w],
                            start=False,
                            stop=(i == NKK16 - 1),
                        )
                    if jj == 4 and prefetch and (
                        prefetch[0], prefetch[1], "h"
                    ) not in enc_tiles:
                        dma_enc(prefetch[0], prefetch[1], "h")
                    if jj == 0 and carry is not None:
                        pb, pci, pw, ps_ps, pesum, pench_t = carry
                        nc.tensor.matmul(
                            ps_ps[:, :pw], ones_col[:], pesum[:, :pw],
                            start=True, stop=True,
                        )
                        finish_chunk(pb, pci, pw, ps_ps, pench_t)
                        carry = None
                    nc.scalar.activation(
                        e_all[:, jj * NT : jj * NT + w], e_ps[:, :w], AF.Tanh,
                        bias=c_sb[:, jj : jj + 1], scale=1.0 / WS,
                    )
                    ev = bpool.tile(
                        [128, NT], dt.float16, tag="ev", bufs=3, name=f"ev{jj}"
                    )
                    nc.vector.tensor_scalar_mul(
                        ev[:, :w], e_all[:, jj * NT : jj * NT + w],
                        v_sb[:, jj : jj + 1],
                    )
                    if acc is None:
                        acc = ev
                    else:
                        nacc = bpool.tile(
                            [128, NT], dt.float16, tag="esum", bufs=3,
                            name=f"esum{jj}",
                        )
                        nc.vector.tensor_add(nacc[:, :w], acc[:, :w], ev[:, :w])
                        acc = nacc
                carry = (b, ci, w, s_ps, acc, ench_t)

            pb, pci, pw, ps_ps, pesum, pench_t = carry
            nc.tensor.matmul(
                ps_ps[:, :pw], ones_col[:], pesum[:, :pw], start=True, stop=True
            )
            finish_chunk(pb, pci, pw, ps_ps, pench_t)

    nc.compile()
    return nc


def _get_nc():
    if "nc" not in _cache:
        import time

        t0 = time.time()
        _cache["nc"] = _build()
        if os.environ.get("KERNEL_TRACE"):
            print(f"[kernel] bass build+compile: {time.time() - t0:.1f} s")
    return _cache["nc"]


def kernel(hidden, encoder_outputs, attn_w, attn_b, v_w):
    from concourse.bass_utils import run_bass_kernel_spmd

    nc = _get_nc()

    hidden = np.asarray(hidden, dtype=np.float32)
    enc = np.asarray(encoder_outputs, dtype=np.float32)
    attn_w = np.asarray(attn_w, dtype=np.float32)
    attn_b = np.asarray(attn_b, dtype=np.float32)
    v_w = np.asarray(v_w, dtype=np.float32)

    w_eT = np.ascontiguousarray(attn_w[:, D:].T) * WS            # [D, H]
    w_kk = w_eT.reshape(KT, 128, H).transpose(1, 0, 2)           # [128, KT, H]
    w8 = np.ascontiguousarray(w_kk).reshape(128, KT * H).astype(F8)
    if NKK16:
        wh = np.ascontiguousarray(w_kk[:, KT - NKK16 :]).reshape(
            128, NKK16 * H
        ).astype(F16)
    c = (hidden @ attn_w[:, :D].T + attn_b).astype(np.float32)   # [1, H]
    c_cols = np.ascontiguousarray(c.reshape(JT, 128).T)          # [128, JT]
    v_cols = np.ascontiguousarray(v_w.reshape(JT, 128).T)

    in_maps = []
    for cidx in range(N_CORES):
        sl = enc[cidx * BPC : (cidx + 1) * BPC]                  # [BPC, S, D]
        rows = []
        for b in range(BPC):
            t0 = 0
            blocks = []
            for wdt in CHUNKS[b]:
                blk = (
                    sl[b, t0 : t0 + wdt]
                    .reshape(wdt, KT, 128)
                    .transpose(2, 1, 0)
                    .reshape(128, KT * wdt)
                )
                blocks.append(blk)
                t0 += wdt
            rows.append(np.concatenate(blocks, axis=1))
        encT2 = np.ascontiguousarray(np.stack(rows))
        m = {
            "enc8": encT2.astype(F8),
            "ench": encT2.astype(F16),
            "w8": w8,
            "c_cols": c_cols,
            "v_cols": v_cols,
        }
        if NKK16:
            m["wh"] = wh
        in_maps.append(m)

    trace = bool(os.environ.get("KERNEL_TRACE"))
    if trace:
        _install_prof_shim()
    res = run_bass_kernel_spmd(
        nc, in_maps, core_ids=list(range(N_CORES)), trace=trace
    )
    if trace:
        _cache["last_exec_time_ns"] = res.exec_time_ns
        print(f"HW exec time: {res.exec_time_ns} ns")

    ctx = np.empty((B, 1, D), dtype=np.float32)
    for cidx in range(N_CORES):
        part = np.asarray(res.results[cidx]["out_part"], dtype=np.float32)
        sums = np.asarray(res.results[cidx]["out_sums"], dtype=np.float32)
        for b in range(BPC):
            nch = len(CHUNKS[b])
            acc = part[b][:, : nch * KT].reshape(128, nch, KT).sum(axis=1)
            ctx[cidx * BPC + b, 0, :] = (
                acc / sums[b][0, :nch].sum()
            ).T.reshape(D)
    return ctx


def _install_prof_shim():
    """antenv.axon_hooks is absent from this image; inject it so
    run_bass_kernel_spmd(trace=True) can capture NTFF profiles."""
    import sys
    import types

    if "antenv.axon_hooks" in sys.modules:
        return
    import antenv

    mod = types.ModuleType("antenv.axon_hooks")
    mod._hook = None
    mod.set_axon_ntff_profile_hook = lambda h: setattr(mod, "_hook", h)
    mod.get_axon_ntff_profile_hook = lambda: mod._hook
    sys.modules["antenv.axon_hooks"] = mod
    antenv.axon_hooks = mod
    try:
        from trn_agent_boot.trn_boot import _ntff_profile_via_ctypes

        mod.set_axon_ntff_profile_hook(
            _ntff_profile_via_ctypes("/opt/axon/libaxon_pjrt.so")
        )
    except Exception:
        pass


# revision 15
# speedup vs baseline: 1.7376x; 1.1038x over previous
"""Bass/Trainium2 kernel for nn_Attention_21354577395789 (fp8 DoubleRow).

Reference computation (B=16, S=2048, H=1024, D=2H=2048):
    h      = broadcast(hidden[1, 2H]) -> [B, S, 2H]
    cat    = concat([h, enc], -1)                    [B, S, 4H]
    energy = tanh(cat @ attn_w.T + attn_b)           [B, S, H]
    scores = energy @ v_w.T                          [B, S, 1]
    attn   = softmax(scores, axis=1)
    ctx    = attn^T @ enc                            [B, 1, 2H]

Algebraic simplifications (as the bf16 baseline):
  * attn_w = [W_h | W_e]; c = hidden @ W_h.T + attn_b is a single [H]
    vector computed on the host.
  * streaming softmax without max subtraction; per-chunk unnormalized
    context partials + exp-sums; final reduction on host.

Precision strategy (the speedup over the bf16 baseline):
  * The O(S*D*H) energy matmul runs in fp8-e4m3 with
    perf_mode=DoubleRow: two k-slices (256 contraction rows) per PE
    pass -> ~1.8x the bf16 matmul rate. W_e is pre-scaled by WS=64 on
    the host (its std 1/64 would land in e4m3's subnormal range);
    the tanh activation applies scale=1/WS to undo it.
  * Optionally the last NKK16 k-tiles run as plain fp16 matmuls into
    the same PSUM accumulation, dialing the quantization error down
    (sim: NKK16=0 -> 1.78e-2, 2 -> 1.68e-2, 4 -> 1.55e-2 vs the 2e-2
    gate) at ~267ns per extra matmul.
  * The context path (DVE) reads a separate fp16 copy of enc, so fp8
    noise never touches the context accumulation.

Engine placement per chunk of S (as baseline): PE energy matmuls +
ones-reduce; ACT tanh/exp; GpSimd broadcast; DVE v-mult tree + context
partials. Data-parallel over B across 8 cores, no collectives.
"""

import os

import numpy as np
import ml_dtypes

B, S, H = 16, 2048, 1024
D = 2 * H
N_CORES = 8
BPC = B // N_CORES  # 2
NT = 512
KT = D // 128       # 16 k-tiles
JT = H // 128       # 8 j-tiles

WS = 64.0           # host-side W_e scale (undone in tanh's scale arg)
NKK16 = 0           # trailing k-tiles computed in fp16 (error dial)

CHUNKS = [[512] * 4 for _ in range(BPC)]
CHUNKS[BPC - 1] = [512, 512, 512, 256, 256]
NCH = max(len(c) for c in CHUNKS)

F8 = ml_dtypes.float8_e4m3
F16 = np.float16

_cache = {}


def _build():
    import concourse.bacc as bacc
    import concourse.tile as tile
    from concourse import mybir

    nc = bacc.Bacc("TRN2", target_bir_lowering=False, debug=False)
    dt = mybir.dt
    DR = mybir.MatmulPerfMode.DoubleRow

    # chunk-major: for chunk ci of batch b (width w, t-offset t0), columns
    # [KT*t0 : KT*(t0+w)] hold block[p, kk*w + t] = enc[b, t0+t, kk*128+p]
    # enc8 feeds the PE (3D tile for the DoubleRow pair APs); ench feeds
    # the DVE/GpSimd context path (2D tile -- 3D APs cost ~+220ns per
    # DVE op in the reshape front-end) and the optional fp16 matmuls.
    enc8_d = nc.declare_dram_parameter(
        "enc8", [BPC, 128, KT * S], dt.float8e4, isOutput=False
    )
    ench_d = nc.declare_dram_parameter(
        "ench", [BPC, 128, KT * S], dt.float16, isOutput=False
    )
    # w8[p, kk*H + jj*128 + j] = WS * w_eT[kk*128 + p, jj*128 + j]
    w8_d = nc.declare_dram_parameter("w8", [128, KT * H], dt.float8e4, isOutput=False)
    if NKK16:
        wh_d = nc.declare_dram_parameter(
            "wh", [128, NKK16 * H], dt.float16, isOutput=False
        )
    c_cols_d = nc.declare_dram_parameter("c_cols", [128, JT], dt.float32, isOutput=False)
    # fp16: used as the stationary operand of the per-jj score matmuls
    v_cols_d = nc.declare_dram_parameter("v_cols", [128, JT], dt.float16, isOutput=False)
    out_part = nc.declare_dram_parameter(
        "out_part", [BPC, 128, NCH * KT], dt.float32, isOutput=True
    )
    out_sums = nc.declare_dram_parameter(
        "out_sums", [BPC, 1, NCH], dt.float32, isOutput=True
    )

    AF = mybir.ActivationFunctionType
    OP = mybir.AluOpType
    NP8 = KT - NKK16          # k-tiles in fp8 (paired for DoubleRow)
    NPAIR = NP8 // 2

    with tile.TileContext(nc) as tc:
        with (
            tc.tile_pool(name="weights", bufs=1) as wpool,
            tc.tile_pool(name="enc8", bufs=3) as e8pool,
            tc.tile_pool(name="ench", bufs=3) as ehpool,
            tc.tile_pool(name="energy", bufs=2) as epool,
            tc.tile_pool(name="perb", bufs=2) as bpool,
            tc.tile_pool(name="psum_e", bufs=5, space="PSUM") as pe_pool,
            tc.tile_pool(name="psum_s", bufs=2, space="PSUM") as ps_pool,
        ):
            # ---- resident weights/constants -----------------------------
            w8_sb = wpool.tile([128, KT, H], dt.float8e4, tag="w8")
            if NKK16:
                wh_sb = wpool.tile([128, NKK16, H], dt.float16, tag="wh")
            c_sb = wpool.tile([128, JT], dt.float32, tag="c")
            v_sb = wpool.tile([128, JT], dt.float16, tag="v")

            def dma_w8(kk, half=None):
                lo = 0 if half != 1 else H // 2
                hi = H if half != 0 else H // 2
                nc.sync.dma_start(
                    w8_sb[:, kk, lo:hi], w8_d.ap()[:, kk * H + lo : kk * H + hi]
                )

            enc_tiles = {}
            offs = [
                [sum(CHUNKS[b][:ci]) for ci in range(len(CHUNKS[b]))]
                for b in range(BPC)
            ]

            def dma_enc(b, ci, which):
                """DMA one chunk of enc8 ('8', 3D tile) or ench ('h', 2D)."""
                w = CHUNKS[b][ci]
                c0 = KT * offs[b][ci]
                src = (enc8_d if which == "8" else ench_d).ap()[b]
                # split per kk-pair so the round-robin queue assignment
                # spreads one chunk's transfer across 8 DMA queues
                if which == "8":
                    t = e8pool.tile(
                        [128, KT, NT], dt.float8e4, tag="enc8", name=f"enc8{b}_{ci}"
                    )
                    for k0 in range(0, KT, 2):
                        nc.sync.dma_start(
                            t[:, k0 : k0 + 2, :w],
                            src[:, c0 + k0 * w : c0 + (k0 + 2) * w],
                        )
                else:
                    # 2D chunk-major [kk*w + t], exactly the DRAM layout
                    t = ehpool.tile(
                        [128, KT * NT], dt.float16, tag="ench", name=f"ench{b}_{ci}"
                    )
                    for k0 in range(0, KT, 2):
                        nc.sync.dma_start(
                            t[:, k0 * w : (k0 + 2) * w],
                            src[:, c0 + k0 * w : c0 + (k0 + 2) * w],
                        )
                enc_tiles[(b, ci, which)] = t

            # startup: constants, then interleave per-kk fp8 weight slices
            # with the first chunk's enc8 pair-blocks so the jj-low matmuls
            # stream kk by kk behind the DMA arrivals.
            nc.sync.dma_start(c_sb[:], c_cols_d.ap()[:])
            nc.sync.dma_start(v_sb[:], v_cols_d.ap()[:])
            first8 = e8pool.tile([128, KT, NT], dt.float8e4, tag="enc8", name="enc80_0")
            src80 = enc8_d.ap()[0]
            for k0 in range(0, KT, 2):
                dma_w8(k0, half=0)
                nc.sync.dma_start(
                    first8[:, k0 : k0 + 2, :],
                    src80[:, k0 * NT : (k0 + 2) * NT],
                )
                dma_w8(k0 + 1, half=0)
            if NKK16:
                nc.sync.dma_start(wh_sb[:, :, :], wh_d.ap()[:, :])
            for kk in range(KT):
                dma_w8(kk, half=1)
            enc_tiles[(0, 0, "8")] = first8
            dma_enc(0, 1, "8")
            dma_enc(0, 0, "h")

            sums_t = {}
            part_t = {}
            for b in range(BPC):
                sums_t[b] = bpool.tile(
                    [1, NCH], dt.float32, tag="sums", name=f"sums{b}"
                )
                part_t[b] = bpool.tile(
                    [128, NCH * KT], dt.float32, tag="part", name=f"part{b}"
                )

            all_chunks = [
                (b, ci) for b in range(BPC) for ci in range(len(CHUNKS[b]))
            ]

            # Scores: per-jj PE matmuls with stationary v_cols[:, jj]
            # ([128,1] fp16, trivial weight load) against the fp16 tanh
            # output, accumulating into s_ps across jj. Each v-MM is
            # issued two jj-blocks late so the PE (in-order queue) never
            # waits on the ACT tanh; the last two v-MMs carry into the
            # next chunk.
            pend = []  # [(s_ps, jj, w)] v-MMs not yet issued

            def finish_chunk(b, ci, w, s_ps, ench_t):
                u_row = bpool.tile([1, NT], dt.float16, tag="urow")
                nc.scalar.activation(
                    u_row[:, :w], s_ps[:, :w], AF.Exp,
                    accum_out=sums_t[b][0:1, ci : ci + 1],
                )
                u_bc = bpool.tile([128, NT], dt.float16, tag="ubc")
                nc.gpsimd.partition_broadcast(u_bc[:, :w], u_row[:, :w])
                for kk in range(KT):
                    eng = nc.vector
                    scratch = bpool.tile([128, NT], dt.bfloat16, tag="scr")
                    eng.scalar_tensor_tensor(
                        out=scratch[:, :w],
                        in0=ench_t[:, kk * w : (kk + 1) * w],
                        scalar=1.0,
                        in1=u_bc[:, :w],
                        op0=OP.mult,
                        op1=OP.mult,
                        accum_out=part_t[b][:, ci * KT + kk : ci * KT + kk + 1],
                    )
                nc.sync.dma_start(
                    out_part.ap()[b][:, ci * KT : (ci + 1) * KT],
                    part_t[b][:, ci * KT : (ci + 1) * KT],
                )
                nch = len(CHUNKS[b])
                if ci == nch - 1:
                    nc.sync.dma_start(
                        out_sums.ap()[b][:, :nch], sums_t[b][0:1, :nch]
                    )

            def issue_vmm(e_all, s_ps, jj, w):
                nc.tensor.matmul(
                    s_ps[:, :w],
                    v_sb[:, jj : jj + 1],
                    e_all[:, jj * NT : jj * NT + w],
                    start=(jj == 0),
                    stop=(jj == JT - 1),
                    skip_group_check=True,
                )

            done = None  # previous chunk's (b, ci, w, s_ps, ench_t)
            for b, ci in all_chunks:
                w = CHUNKS[b][ci]
                for which in ("8", "h"):
                    if (b, ci, which) not in enc_tiles:
                        dma_enc(b, ci, which)
                enc8_t = enc_tiles.pop((b, ci, "8"))
                ench_t = enc_tiles.pop((b, ci, "h"))
                # prefetch: enc8 two chunks ahead, ench one ahead (at jj=2)
                nch = len(CHUNKS[b])
                nxt = [(b, c2) for c2 in range(ci + 1, nch)] + [
                    (b2, c2)
                    for b2 in range(b + 1, BPC)
                    for c2 in range(len(CHUNKS[b2]))
                ]
                for pb, pc in nxt[:2]:
                    if (pb, pc, "8") not in enc_tiles:
                        dma_enc(pb, pc, "8")

                s_ps = ps_pool.tile(
                    [1, NT], dt.float32, tag="sps", name=f"sps{b}_{ci}"
                )
                e_all = epool.tile(
                    [128, JT * NT], dt.float16, tag="eall", name=f"eall{b}_{ci}"
                )
                for jj in range(JT):
                    e_ps = pe_pool.tile([128, NT], dt.float32, tag="eps")
                    for pk in range(NPAIR):
                        kk0 = 2 * pk
                        nc.tensor.matmul(
                            e_ps[:, :w],
                            w8_sb[:, kk0 : kk0 + 2, jj * 128 : (jj + 1) * 128],
                            enc8_t[:, kk0 : kk0 + 2, :w],
                            start=(pk == 0),
                            stop=(pk == NPAIR - 1 and NKK16 == 0),
                            perf_mode=DR,
                        )
                    for i in range(NKK16):
                        kk = NP8 + i
                        nc.tensor.matmul(
                            e_ps[:, :w],
                            wh_sb[:, i, jj * 128 : (jj + 1) * 128],
                            ench_t[:, kk * w : (kk + 1) * w],
                            start=False,
                            stop=(i == NKK16 - 1),
                        )
                    if jj == 2 and nxt and (nxt[0][0], nxt[0][1], "h") not in enc_tiles:
                        dma_enc(nxt[0][0], nxt[0][1], "h")
                    # drain deferred v-MMs, keeping 2 in flight behind the
                    # DR stream so the in-order PE never waits on a tanh
                    while len(pend) > 2:
                        issue_vmm(*pend.pop(0))
                    if pend and pend[0][2] == JT - 1:
                        issue_vmm(*pend.pop(0))
                        pdone, done = done, None
                        finish_chunk(*pdone)
                    nc.scalar.activation(
                        e_all[:, jj * NT : jj * NT + w], e_ps[:, :w], AF.Tanh,
                        bias=c_sb[:, jj : jj + 1], scale=1.0 / WS,
                    )
                    pend.append((e_all, s_ps, jj, w))
                done = (b, ci, w, s_ps, ench_t)

            while pend:
                issue_vmm(*pend.pop(0))
            finish_chunk(*done)

    nc.compile()
    return nc


def _get_nc():
    if "nc" not in _cache:
        import time

        t0 = time.time()
        _cache["nc"] = _build()
        if os.environ.get("KERNEL_TRACE"):
            print(f"[kernel] bass build+compile: {time.time() - t0:.1f} s")
    return _cache["nc"]


def kernel(hidden, encoder_outputs, attn_w, attn_b, v_w):
    from concourse.bass_utils import run_bass_kernel_spmd

    nc = _get_nc()

    hidden = np.asarray(hidden, dtype=np.float32)
    enc = np.asarray(encoder_outputs, dtype=np.float32)
    attn_w = np.asarray(attn_w, dtype=np.float32)
    attn_b = np.asarray(attn_b, dtype=np.float32)
    v_w = np.asarray(v_w, dtype=np.float32)

    w_eT = np.ascontiguousarray(attn_w[:, D:].T) * WS            # [D, H]
    w_kk = w_eT.reshape(KT, 128, H).transpose(1, 0, 2)           # [128, KT, H]
    w8 = np.ascontiguousarray(w_kk).reshape(128, KT * H).astype(F8)
    if NKK16:
        wh = np.ascontiguousarray(w_kk[:, KT - NKK16 :]).reshape(
            128, NKK16 * H
        ).astype(F16)
    c = (hidden @ attn_w[:, :D].T + attn_b).astype(np.float32)   # [1, H]
    c_cols = np.ascontiguousarray(c.reshape(JT, 128).T)          # [128, JT]
    v_cols = np.ascontiguousarray(v_w.reshape(JT, 128).T).astype(F16)

    in_maps = []
    for cidx in range(N_CORES):
        sl = enc[cidx * BPC : (cidx + 1) * BPC]                  # [BPC, S, D]
        rows = []
        for b in range(BPC):
            t0 = 0
            blocks = []
            for wdt in CHUNKS[b]:
                blk = (
                    sl[b, t0 : t0 + wdt]
                    .reshape(wdt, KT, 128)
                    .transpose(2, 1, 0)
                    .reshape(128, KT * wdt)
                )
                blocks.append(blk)
                t0 += wdt
            rows.append(np.concatenate(blocks, axis=1))
        encT2 = np.ascontiguousarray(np.stack(rows))
        m = {
            "enc8": encT2.astype(F8),
            "ench": encT2.astype(F16),
            "w8": w8,
            "c_cols": c_cols,
            "v_cols": v_cols,
        }
        if NKK16:
            m["wh"] = wh
        in_maps.append(m)

    trace = bool(os.environ.get("KERNEL_TRACE"))
    if trace:
        _install_prof_shim()
    res = run_bass_kernel_spmd(
        nc, in_maps, core_ids=list(range(N_CORES)), trace=trace
    )
    if trace:
        _cache["last_exec_time_ns"] = res.exec_time_ns
        print(f"HW exec time: {res.exec_time_ns} ns")

    ctx = np.empty((B, 1, D), dtype=np.float32)
    for cidx in range(N_CORES):
        part = np.asarray(res.results[cidx]["out_part"], dtype=np.float32)
        sums = np.asarray(res.results[cidx]["out_sums"], dtype=np.float32)
        for b in range(BPC):
            nch = len(CHUNKS[b])
            acc = part[b][:, : nch * KT].reshape(128, nch, KT).sum(axis=1)
            ctx[cidx * BPC + b, 0, :] = (
                acc / sums[b][0, :nch].sum()
            ).T.reshape(D)
    return ctx


def _install_prof_shim():
    """antenv.axon_hooks is absent from this image; inject it so
    run_bass_kernel_spmd(trace=True) can capture NTFF profiles."""
    import sys
    import types

    if "antenv.axon_hooks" in sys.modules:
        return
    import antenv

    mod = types.ModuleType("antenv.axon_hooks")
    mod._hook = None
    mod.set_axon_ntff_profile_hook = lambda h: setattr(mod, "_hook", h)
    mod.get_axon_ntff_profile_hook = lambda: mod._hook
    sys.modules["antenv.axon_hooks"] = mod
    antenv.axon_hooks = mod
    try:
        from trn_agent_boot.trn_boot import _ntff_profile_via_ctypes

        mod.set_axon_ntff_profile_hook(
            _ntff_profile_via_ctypes("/opt/axon/libaxon_pjrt.so")
        )
    except Exception:
        pass
